# revision 1
# baseline (speedup 1.0000x reference)
"""Trainium2 Bass kernel for nn_Net_83794811945603 (3-layer GraphSAGE, mean agg).

Computation (N=50000 nodes, E=800000 edges):
    h0 = x @ W_map + b_map                                  [N,128]
    h1 = relu(mean_agg(h0) @ Wl1 + bl1 + h0 @ Wr1)          [N,128]
    h2 = relu(mean_agg(h1) @ Wl2 + bl2 + h1 @ Wr2)          [N,256]
    out = log_softmax(mean_agg(h2) @ Wl3 + bl3 + h2 @ Wr3)  [N,40]
where mean_agg(h)[i] = mean over edges (s->i) of h[s].

Strategy (8 NeuronCores, SPMD):
  - Nodes sharded row-wise: core c owns nodes [c*6250, (c+1)*6250) and all
    edges whose dst lands there.  Weights replicated.
  - Per layer: full previous-layer features live in DRAM (AllGather output).
    Edge-gather h[src] rows via SWDGE dma_gather (int16 indices; the node
    table is split in two 25000-row halves so indices fit in int16).
  - Segment-sum by dst via TensorE one-hot matmuls: for each 128-edge chunk,
    aggT += G_chunk^T @ S_chunk where S[e, j] = (dst_local[e] == j), built on
    VectorE with a broadcast is_equal against an iota row.  Mean = aggT *
    inv_deg (free-axis broadcast multiply with a host-replicated table).
  - Layer 3 aggregates z3 = h2 @ Wl3 (40->pad 64 wide) instead of h2
    (256 wide): segment_mean commutes with the right-matmul, 4x less gather
    traffic.
  - All dense matmuls run in the transposed layout hT = [feat, node] so the
    contraction dim always sits on partitions; node-major copies (needed for
    the row-gather and the output) are produced with PE transposes.
"""

import os
import sys

sys.path.insert(0, "/opt/trn_rl_repo")

import numpy as np
import ml_dtypes

import concourse.bass as bass
import concourse.bacc as bacc
import concourse.tile as tile
import concourse.mybir as mybir
from concourse.bass_utils import run_bass_kernel_spmd
from concourse.library_config import mlp

F32 = mybir.dt.float32
BF16 = mybir.dt.bfloat16
I16 = mybir.dt.int16
I32 = mybir.dt.int32

N = 50000
E = 800000
F_IN = 500
F_IN_PAD = 512
D = 128            # ID_DIM == HID
D2 = 256           # 2*HID
NCLS = 40
NCLS_PAD = 128
NCORES = 8
NLOC = N // NCORES          # 6250
NT = (NLOC + 127) // 128    # 49 dst tiles per core
NLOC_PAD = NT * 128         # 6272
HALF = N // 2               # int16 index split
KSUP = 2                    # dst tiles per gather "supertile"
NSUP = (NT + KSUP - 1) // KSUP          # 13
SUP_TILES = [min(KSUP, NT - T * KSUP) for T in range(NSUP)]  # [4]*12 + [1]


def _ts(i, size=128):
    return slice(i * size, (i + 1) * size)


class _StopBuild(Exception):
    pass


def build_program(CH, build_stage=None):
    """Build the SPMD Bass program. CH = 128-edge chunks per (tile, half).
    build_stage (debug): 0 = stage0+AG only, 1 = +L1+AG, 2 = +L2+AG, 3 = full."""
    if build_stage is None:
        build_stage = int(os.environ.get("KERNEL_BUILD_STAGE", "3"))
    nc = bacc.Bacc("TRN2", target_bir_lowering=False, debug=False,
                   num_devices=NCORES, num_swdge_queues=4)

    SEG = CH * 128                        # index slots per (tile, half)
    NP = (SEG + 1023) // 1024             # gather pieces per (tile, half)
    # segment column offsets in dl (units of CH cols) / idx (units of CH*8)
    seg_tiles = []                        # (T, h) -> tiles in segment
    for T in range(NSUP):
        for h in (0, 1):
            seg_tiles.append(SUP_TILES[T])
    seg_dl_off = np.concatenate([[0], np.cumsum([t * CH for t in seg_tiles])])
    seg_ix_off = np.concatenate([[0], np.cumsum([t * CH * 8 for t in seg_tiles])])
    DL_COLS = int(seg_dl_off[-1])         # NT*2*CH
    IX_COLS = int(seg_ix_off[-1])         # NT*2*CH*8

    def seg_idx(T, h):
        return T * 2 + h

    # ---- I/O ----
    xT = nc.dram_tensor("xT", [F_IN_PAD, NLOC_PAD], BF16, kind="ExternalInput")
    idx_d = nc.dram_tensor("idx", [128, IX_COLS], I16, kind="ExternalInput")
    dl_d = nc.dram_tensor("dl", [128, DL_COLS], F32, kind="ExternalInput")
    invdeg_d = nc.dram_tensor("invdeg", [128, NLOC_PAD], F32, kind="ExternalInput")
    iota_d = nc.dram_tensor("iota", [128, 128], F32, kind="ExternalInput")
    ident_d = nc.dram_tensor("ident", [128, 128], BF16, kind="ExternalInput")
    ident32_d = nc.dram_tensor("ident32", [128, 128], F32, kind="ExternalInput")
    wmap_d = nc.dram_tensor("wmap", [128, F_IN_PAD], BF16, kind="ExternalInput")
    bmap_d = nc.dram_tensor("bmap", [128, 1], F32, kind="ExternalInput")
    wl1_d = nc.dram_tensor("wl1", [128, D], BF16, kind="ExternalInput")
    wr1_d = nc.dram_tensor("wr1", [128, D], BF16, kind="ExternalInput")
    bl1_d = nc.dram_tensor("bl1", [128, 1], F32, kind="ExternalInput")
    wl2_d = nc.dram_tensor("wl2", [128, D2], BF16, kind="ExternalInput")
    wr2_d = nc.dram_tensor("wr2", [128, D2], BF16, kind="ExternalInput")
    bl2_d = nc.dram_tensor("bl2", [128, 2], F32, kind="ExternalInput")
    wl3_d = nc.dram_tensor("wl3", [128, 2 * NCLS_PAD], BF16, kind="ExternalInput")
    wr3_d = nc.dram_tensor("wr3", [128, 2 * NCLS_PAD], BF16, kind="ExternalInput")
    bl3_d = nc.dram_tensor("bl3", [NCLS_PAD, 1], F32, kind="ExternalInput")
    cnt_d = nc.dram_tensor("cnt", [1, NT * 2 * NP], I32, kind="ExternalInput")
    out_d = nc.dram_tensor("out", [NLOC, NCLS], F32, kind="ExternalOutput")

    # internal DRAM
    h0loc = nc.dram_tensor("h0loc", [NLOC, D], BF16)
    h1loc = nc.dram_tensor("h1loc", [NLOC, D], BF16)
    z3loc = nc.dram_tensor("z3loc", [NLOC, NCLS_PAD], BF16)
    h0full = nc.dram_tensor("h0full", [N, D], BF16, addr_space="Shared")
    h1full = nc.dram_tensor("h1full", [N, D], BF16, addr_space="Shared")
    z3full = nc.dram_tensor("z3full", [N, NCLS_PAD], BF16, addr_space="Shared")

    groups = [list(range(NCORES))]

    from contextlib import ExitStack
    ctx_regs = ExitStack()
    with tile.TileContext(nc) as tc:
        with (
            tc.tile_pool(name="const", bufs=1) as cp,
            tc.tile_pool(name="hres", bufs=1) as hp,
            tc.tile_pool(name="gat", bufs=6) as gp,
            tc.tile_pool(name="sone", bufs=5) as sp,
            tc.tile_pool(name="work", bufs=3) as wp,
            tc.tile_pool(name="xin", bufs=4) as xp,
            tc.tile_pool(name="pa", bufs=2 * KSUP, space="PSUM") as pa,
            tc.tile_pool(name="po", bufs=2, space="PSUM") as po,
            tc.tile_pool(name="pt", bufs=2, space="PSUM") as pt,
        ):
            # ---- load constants ----
            idx_sb = cp.tile([128, IX_COLS], I16)
            dl_sb = cp.tile([128, DL_COLS], F32)
            invdeg = cp.tile([128, NLOC_PAD], F32)
            iota = cp.tile([128, 128], F32)
            ident = cp.tile([128, 128], BF16)
            ident32 = cp.tile([128, 128], F32)
            wmap = cp.tile([128, F_IN_PAD], BF16)
            bmap = cp.tile([128, 1], F32)
            wl1 = cp.tile([128, D], BF16)
            wr1 = cp.tile([128, D], BF16)
            bl1 = cp.tile([128, 1], F32)
            wl2 = cp.tile([128, D2], BF16)
            wr2 = cp.tile([128, D2], BF16)
            bl2 = cp.tile([128, 2], F32)
            wl3 = cp.tile([128, 2 * NCLS_PAD], BF16)
            wr3 = cp.tile([128, 2 * NCLS_PAD], BF16)
            bl3 = cp.tile([NCLS_PAD, 1], F32)
            cnt_sb = cp.tile([1, NT * 2 * NP], I32)
            for sb_t, dr in [(idx_sb, idx_d), (dl_sb, dl_d), (invdeg, invdeg_d),
                             (iota, iota_d), (ident, ident_d), (ident32, ident32_d), (wmap, wmap_d),
                             (bmap, bmap_d), (wl1, wl1_d), (wr1, wr1_d),
                             (bl1, bl1_d), (wl2, wl2_d), (wr2, wr2_d),
                             (bl2, bl2_d), (wl3, wl3_d), (wr3, wr3_d),
                             (bl3, bl3_d), (cnt_sb, cnt_d)]:
                nc.scalar.dma_start(out=sb_t[:], in_=dr[:])

            # persistent transposed activations (tags share slots over time)
            h0T = hp.tile([128, NLOC_PAD], BF16, tag="hA")
            h1T = hp.tile([128, NLOC_PAD], BF16, tag="hB")

            # ---- stage 0: h0T = W_map^T @ xT + b_map ----
            NH = NLOC_PAD // 2
            for half_n in (0, 1):
                slabs = [xp.tile([128, NH], BF16, tag="xs",
                                 name=f"xs{half_n}_{k}") for k in range(4)]
                for k in range(4):
                    nc.sync.dma_start(out=slabs[k][:],
                                      in_=xT[_ts(k), half_n * NH:(half_n + 1) * NH])
                nh_chunks = [(i * 512, min(512, NH - i * 512))
                             for i in range((NH + 511) // 512)]
                for n0, nw in nh_chunks:
                    ps = po.tile([128, 512], F32, tag="po")
                    for k in range(4):
                        nc.tensor.matmul(ps[:, 0:nw], wmap[:, _ts(k)],
                                         slabs[k][:, n0:n0 + nw],
                                         start=(k == 0), stop=(k == 3))
                    g0 = half_n * NH + n0
                    nc.vector.tensor_scalar(out=h0T[:, g0:g0 + nw],
                                            in0=ps[:, 0:nw],
                                            scalar1=bmap[:, 0:1], scalar2=None,
                                            op0=mybir.AluOpType.add)
            # node-major h0 -> DRAM
            for t in range(NT):
                ptr = pt.tile([128, 128], BF16, tag="pt")
                nc.tensor.transpose(ptr[:], h0T[:, _ts(t)], ident[:])
                nm = wp.tile([128, 128], BF16, tag="nm")
                nc.scalar.copy(nm[:], ptr[:])
                rows = min(128, NLOC - t * 128)
                nc.sync.dma_start(out=h0loc[t * 128: t * 128 + rows, :],
                                  in_=nm[0:rows, :])
            for q in (0, 1):
                nc.gpsimd.collective_compute(
                    "AllGather", mybir.AluOpType.bypass, replica_groups=groups,
                    ins=[h0loc[q * (NLOC // 2):(q + 1) * (NLOC // 2), :]],
                    outs=[h0full[q * HALF:(q + 1) * HALF, :]])

            # zero the gather-pool slots once: reg-skipped trailing slots
            # are never written, and 0 * S keeps matmul inputs finite
            for zi in range(6):
                gz = gp.tile([128, KSUP * CH, 128], BF16, tag="g",
                             name=f"gz{zi}")
                nc.vector.memset(gz[:], 0.0)

            # ---- generic edge aggregation ----
            self_queue = [0]
            SEGSZ = CH * 128
            pieces = []
            p0 = 0
            while p0 < SEGSZ:
                pieces.append((p0, min(1024, SEGSZ - p0)))
                p0 += 1024


            def aggregate(T, h, src_full, elem, pa_tiles, first, last):
                """Gather + one-hot matmuls for supertile T, half h.
                pa_tiles[ti]: per-dst-tile psum bank (one accumulation group
                per bank -- PSUM start zeroes a whole 2KB zero-region, so
                groups must not share banks)."""
                ntl = SUP_TILES[T]
                g = gp.tile([128, KSUP * CH, elem], BF16, tag="g")
                s = seg_idx(T, h)
                io0 = int(seg_ix_off[s])
                SEG = CH * 128
                # <=1024-idx single-packet gathers over the whole segment,
                # rotating queues for parallel Q7 descriptor generation
                nidx = ntl * SEG
                done = 0
                while done < nidx:
                    n = min(1024, nidx - done)
                    nc.gpsimd.dma_gather(
                        g[:, done // 128:(done + n) // 128, :],
                        src_full[h * HALF:(h + 1) * HALF, :],
                        idx_sb[:, io0 + done // 16: io0 + (done + n) // 16],
                        n, n, elem,
                        single_packet=True, queue_num=self_queue[0])
                    self_queue[0] = (self_queue[0] + 1) % 4
                    done += n
                d0 = int(seg_dl_off[s])
                sone = sp.tile([128, KSUP * CH, 128], BF16, tag="S")
                io_b = iota[:].rearrange("p (o j) -> p o j", o=1) \
                    .broadcast_to([128, ntl * CH, 128])
                dl_b = dl_sb[:, d0:d0 + ntl * CH] \
                    .rearrange("p (c o) -> p c o", o=1) \
                    .broadcast_to([128, ntl * CH, 128])
                nc.vector.tensor_tensor(out=sone[:, 0:ntl * CH, :], in0=io_b,
                                        in1=dl_b, op=mybir.AluOpType.is_equal)
                for ti in range(ntl):
                    for cc in range(CH):
                        j = ti * CH + cc
                        nc.tensor.matmul(
                            pa_tiles[ti][0:elem, :],
                            g[:, j, :], sone[:, j, :],
                            start=(first and cc == 0), stop=(last and cc == CH - 1))

            def layer_loop(src_full, elem, tile_body, mean_dt=BF16):
                """For each supertile: gather both halves, then per-tile body.
                tile_body(t, mean_sb) consumes the inv-deg-scaled aggT."""
                for T in range(NSUP):
                    ntl = SUP_TILES[T]
                    pa_tiles = [pa.tile([128, 128], F32, tag="pa",
                                        name=f"pa_{T}_{i}")
                                for i in range(ntl)]
                    aggregate(T, 0, src_full, elem, pa_tiles, True, False)
                    aggregate(T, 1, src_full, elem, pa_tiles, False, True)
                    for ti in range(ntl):
                        t = T * KSUP + ti
                        mean = wp.tile([128, 128], mean_dt, tag="mean")
                        nc.vector.tensor_tensor(
                            out=mean[0:elem, :], in0=pa_tiles[ti][0:elem, :],
                            in1=invdeg[0:elem, _ts(t)],
                            op=mybir.AluOpType.mult)
                        tile_body(t, mean)

            # ---- layer 1 ----
            def l1_body(t, mean):
                rows = min(128, NLOC - t * 128)
                p1 = po.tile([128, 128], F32, tag="po")
                nc.tensor.matmul(p1[:], wl1[:], mean[:], start=True, stop=False)
                nc.tensor.matmul(p1[:], wr1[:], h0T[:, _ts(t)],
                                 start=False, stop=True)
                nc.scalar.activation(out=h1T[:, _ts(t)], in_=p1[:],
                                     func=mybir.ActivationFunctionType.Relu,
                                     bias=bl1[:, 0:1], scale=1.0)
                ptr = pt.tile([128, 128], BF16, tag="pt")
                nc.tensor.transpose(ptr[:], h1T[:, _ts(t)], ident[:])
                nm = wp.tile([128, 128], BF16, tag="nm")
                nc.scalar.copy(nm[:], ptr[:])
                nc.sync.dma_start(out=h1loc[t * 128: t * 128 + rows, :],
                                  in_=nm[0:rows, :])

            if build_stage >= 1:
                layer_loop(h0full, D, l1_body)
                for q in (0, 1):
                    nc.gpsimd.collective_compute(
                        "AllGather", mybir.AluOpType.bypass,
                        replica_groups=groups,
                        ins=[h1loc[q * (NLOC // 2):(q + 1) * (NLOC // 2), :]],
                        outs=[h1full[q * HALF:(q + 1) * HALF, :]])

            # ---- layer 2 (+ z3 projection) ----
            h2T0 = hp.tile([128, NLOC_PAD], BF16, tag="hA")  # reuses h0T slot
            h2T1 = hp.tile([128, NLOC_PAD], BF16, tag="hC")

            def l2_body(t, mean):
                rows = min(128, NLOC - t * 128)
                for hh, (h2T_h, wcol) in enumerate(((h2T0, _ts(0)),
                                                    (h2T1, _ts(1)))):
                    p2 = po.tile([128, 128], F32, tag="po")
                    nc.tensor.matmul(p2[:], wl2[:, wcol], mean[:],
                                     start=True, stop=False)
                    nc.tensor.matmul(p2[:], wr2[:, wcol], h1T[:, _ts(t)],
                                     start=False, stop=True)
                    nc.scalar.activation(
                        out=h2T_h[:, _ts(t)], in_=p2[:],
                        func=mybir.ActivationFunctionType.Relu,
                        bias=bl2[:, hh:hh + 1], scale=1.0)
                # z3 = h2 @ Wl3 (transposed: z3T = Wl3^T @ h2T), 64-padded
                pz = po.tile([128, 128], F32, tag="po")
                nc.tensor.matmul(pz[0:NCLS_PAD, :], wl3[:, 0:NCLS_PAD],
                                 h2T0[:, _ts(t)], start=True, stop=False)
                nc.tensor.matmul(pz[0:NCLS_PAD, :],
                                 wl3[:, NCLS_PAD:2 * NCLS_PAD],
                                 h2T1[:, _ts(t)], start=False, stop=True)
                zt = wp.tile([NCLS_PAD, 128], BF16, tag="zt")
                nc.scalar.copy(zt[:], pz[0:NCLS_PAD, :])
                ptz = pt.tile([128, 128], BF16, tag="pt")
                nc.tensor.transpose(ptz[:, 0:NCLS_PAD], zt[:],
                                    ident[0:NCLS_PAD, 0:NCLS_PAD])
                nmz = wp.tile([128, 128], BF16, tag="nm")
                nc.scalar.copy(nmz[:, 0:NCLS_PAD], ptz[:, 0:NCLS_PAD])
                nc.sync.dma_start(out=z3loc[t * 128: t * 128 + rows, :],
                                  in_=nmz[0:rows, 0:NCLS_PAD])

            if build_stage >= 2:
                layer_loop(h1full, D, l2_body)
                for q in (0, 1):
                    nc.gpsimd.collective_compute(
                        "AllGather", mybir.AluOpType.bypass,
                        replica_groups=groups,
                        ins=[z3loc[q * (NLOC // 2):(q + 1) * (NLOC // 2), :]],
                        outs=[z3full[q * HALF:(q + 1) * HALF, :]])

            # ---- layer 3 + log_softmax ----
            def l3_body(t, mean):
                rows = min(128, NLOC - t * 128)
                p3 = po.tile([128, 128], F32, tag="po")
                nc.tensor.matmul(p3[0:NCLS_PAD, :], wr3[:, 0:NCLS_PAD],
                                 h2T0[:, _ts(t)], start=True, stop=False)
                nc.tensor.matmul(p3[0:NCLS_PAD, :],
                                 wr3[:, NCLS_PAD:2 * NCLS_PAD],
                                 h2T1[:, _ts(t)], start=False, stop=True)
                # mean (already inv-deg scaled) + wr3 term + bias
                W64 = 64
                comb = wp.tile([W64, 128], F32, tag="comb")
                nc.vector.tensor_tensor(out=comb[:], in0=mean[0:W64, :],
                                        in1=p3[0:W64, :],
                                        op=mybir.AluOpType.add)
                # bias add while still class-major (per-partition bias on ACT)
                comb2 = wp.tile([W64, 128], F32, tag="comb2")
                nc.scalar.activation(out=comb2[:], in_=comb[:],
                                     func=mybir.ActivationFunctionType.Identity,
                                     bias=bl3[0:W64, 0:1], scale=1.0)
                ptf = pt.tile([128, 128], F32, tag="pt")
                nc.tensor.transpose(ptf[:, 0:W64], comb2[:],
                                    ident32[0:W64, 0:W64])
                # log_softmax over the 40 valid class columns (ACT-heavy)
                xm = wp.tile([128, 1], F32, tag="xm")
                nc.vector.tensor_reduce(out=xm[:], in_=ptf[:, 0:NCLS],
                                        axis=mybir.AxisListType.X,
                                        op=mybir.AluOpType.max, negate=True)
                tt = wp.tile([128, NCLS], F32, tag="tt")
                nc.scalar.activation(out=tt[:], in_=ptf[:, 0:NCLS],
                                     func=mybir.ActivationFunctionType.Identity,
                                     bias=xm[:, 0:1], scale=1.0)
                ex = wp.tile([128, NCLS], F32, tag="ex")
                ssum = wp.tile([128, 1], F32, tag="ssum")
                nc.scalar.activation(out=ex[:], in_=tt[:],
                                     func=mybir.ActivationFunctionType.Exp,
                                     accum_out=ssum[:])
                lse = wp.tile([128, 1], F32, tag="lse")
                nc.scalar.activation(out=lse[:], in_=ssum[:],
                                     func=mybir.ActivationFunctionType.Ln)
                lsn = wp.tile([128, 1], F32, tag="lsn")
                nc.scalar.mul(lsn[:], lse[:], -1.0)
                fin = wp.tile([128, NCLS], F32, tag="fin")
                nc.scalar.activation(out=fin[:], in_=tt[:],
                                     func=mybir.ActivationFunctionType.Identity,
                                     bias=lsn[:, 0:1], scale=1.0)
                nc.sync.dma_start(out=out_d[t * 128: t * 128 + rows, :],
                                  in_=fin[0:rows, :])

            if build_stage >= 3:
                layer_loop(z3full, NCLS_PAD, l3_body, mean_dt=F32)

    nc.compile()
    return nc


# ---------------- host side ----------------

def _pack_idx_segment(vals: np.ndarray) -> np.ndarray:
    """[L] int16 -> [128, L//16]: slot i -> [i % 16, i // 16], x8 replicated."""
    L = vals.shape[0]
    arr = vals.reshape(L // 16, 16).T  # [16, L//16]
    return np.tile(arr, (8, 1))


def prepare_inputs(x, edge_index, W_map, b_map, Wl1, bl1, Wr1, Wl2, bl2, Wr2,
                   Wl3, bl3, Wr3):
    src = np.asarray(edge_index[0], dtype=np.int64)
    dst = np.asarray(edge_index[1], dtype=np.int64)

    core = dst // NLOC
    local = dst - core * NLOC
    t_loc = local >> 7
    dloc = local & 127
    # chunked-AG table layout: chunk q holds all cores' local rows
    # [q*3125, (q+1)*3125): position = c_src*3125 + (r - q*3125)
    c_src = src // NLOC
    r_src = src - c_src * NLOC
    half = (r_src >= NLOC // 2).astype(np.int64)
    idx16 = (c_src * (NLOC // 2) + (r_src - half * (NLOC // 2))).astype(np.int16)

    T_sup = t_loc // KSUP
    ti = t_loc - T_sup * KSUP

    # fine group for slot assignment: (core, supertile, half, tile_in_sup)
    fine = ((core * NSUP + T_sup) * 2 + half) * KSUP + ti
    NFINE = NCORES * NSUP * 2 * KSUP
    counts = np.bincount(fine, minlength=NFINE)
    CH = int(np.ceil(counts.max() / 128))
    SEG = CH * 128

    order = np.argsort(fine, kind="stable")
    fine_s = fine[order]
    offs = np.concatenate([[0], np.cumsum(counts)])
    pos = np.arange(E) - np.repeat(offs[:-1], counts)

    # flat slot layout per core: segments (T, h) with ti-major inner blocks,
    # but careful: supertile T has SUP_TILES[T] tiles; fine group (T, h, ti)
    # occupies slots [seg_base(T,h) + ti*SEG, ...+counts).
    seg_tiles = []
    for T in range(NSUP):
        for h in (0, 1):
            seg_tiles.append(SUP_TILES[T])
    seg_slot_off = np.concatenate([[0], np.cumsum([t * SEG for t in seg_tiles])])
    SLTOT = int(seg_slot_off[-1])  # slots per core

    # fine group -> global slot base
    fine_base = np.zeros(NFINE, dtype=np.int64)
    for c in range(NCORES):
        for T in range(NSUP):
            for h in (0, 1):
                s = T * 2 + h
                for ti_ in range(SUP_TILES[T]):
                    f = ((c * NSUP + T) * 2 + h) * KSUP + ti_
                    fine_base[f] = c * SLTOT + seg_slot_off[s] + ti_ * SEG

    slot = fine_base[fine_s] + pos  # global slot per sorted edge

    big_idx = np.full(NCORES * SLTOT, 0, dtype=np.int16)
    big_dl = np.full(NCORES * SLTOT, 999.0, dtype=np.float32)
    big_idx[slot] = idx16[order]
    big_dl[slot] = dloc[order].astype(np.float32)
    big_idx = big_idx.reshape(NCORES, SLTOT)
    big_dl = big_dl.reshape(NCORES, SLTOT)

    # per-(core, tile, half, piece) valid counts; ensure >=1 valid per piece
    SEGSZ = CH * 128
    pieces = []
    p0 = 0
    while p0 < SEGSZ:
        pieces.append((p0, min(1024, SEGSZ - p0)))
        p0 += 1024
    NP = len(pieces)
    cnts = np.zeros((NCORES, NT * 2 * NP), dtype=np.int32)
    fine_v = counts  # per fine group (c,T,h,ti)
    for c in range(NCORES):
        for T in range(NSUP):
            for h in (0, 1):
                sg = T * 2 + h
                for ti_ in range(SUP_TILES[T]):
                    f = ((c * NSUP + T) * 2 + h) * KSUP + ti_
                    v = int(fine_v[f])
                    t_glob = T * KSUP + ti_
                    base = c * SLTOT + int(seg_slot_off[sg]) + ti_ * SEGSZ
                    for pi, (q0, plen) in enumerate(pieces):
                        cnts[c, (t_glob * 2 + h) * NP + pi] = plen

    # degrees
    cnt = np.bincount(dst, minlength=N).astype(np.float32)
    inv = 1.0 / np.maximum(cnt, 1.0)

    # weights (shared)
    BF = ml_dtypes.bfloat16
    Wmap_pad = np.zeros((F_IN_PAD, 128), np.float32)
    Wmap_pad[0:F_IN] = W_map
    wmap_kt = np.concatenate([Wmap_pad[_ts(k)] for k in range(4)], axis=1)
    Wl3_pad = np.zeros((D2, NCLS_PAD), np.float32)
    Wl3_pad[:, 0:NCLS] = Wl3
    wl3_kt = np.concatenate([Wl3_pad[_ts(k)] for k in range(2)], axis=1)
    Wr3_pad = np.zeros((D2, NCLS_PAD), np.float32)
    Wr3_pad[:, 0:NCLS] = Wr3
    wr3_kt = np.concatenate([Wr3_pad[_ts(k)] for k in range(2)], axis=1)
    bl3_pad = np.zeros((NCLS_PAD, 1), np.float32)
    bl3_pad[0:NCLS, 0] = bl3

    shared = {
        "iota": np.ascontiguousarray(
            np.tile(np.arange(128, dtype=np.float32), (128, 1))),
        "ident": np.eye(128, dtype=np.float32).astype(BF),
        "ident32": np.eye(128, dtype=np.float32),
        "wmap": np.ascontiguousarray(wmap_kt).astype(BF),
        "bmap": np.ascontiguousarray(b_map.reshape(128, 1)),
        "wl1": np.ascontiguousarray(Wl1).astype(BF),
        "wr1": np.ascontiguousarray(Wr1).astype(BF),
        "bl1": np.ascontiguousarray(bl1.reshape(128, 1)),
        "wl2": np.ascontiguousarray(Wl2).astype(BF),
        "wr2": np.ascontiguousarray(Wr2).astype(BF),
        "bl2": np.ascontiguousarray(bl2.reshape(2, 128).T),
        "wl3": np.ascontiguousarray(wl3_kt).astype(BF),
        "wr3": np.ascontiguousarray(wr3_kt).astype(BF),
        "bl3": bl3_pad,
    }

    in_maps = []
    for c in range(NCORES):
        xT_pad = np.zeros((F_IN_PAD, NLOC_PAD), np.float32)
        xT_pad[0:F_IN, 0:NLOC] = x[c * NLOC:(c + 1) * NLOC].T
        xT_pad = xT_pad.astype(ml_dtypes.bfloat16)

        # idx layout: per (T,h) segment packed independently, concat cols
        seg_cols = []
        dl_cols = []
        for T in range(NSUP):
            for h in (0, 1):
                s = T * 2 + h
                a, b = int(seg_slot_off[s]), int(seg_slot_off[s + 1])
                vals = big_idx[c, a:b]
                seg_cols.append(_pack_idx_segment(vals))
                dls = big_dl[c, a:b].reshape(-1, 128).T  # [128, tiles*CH]
                dl_cols.append(dls)
        idx_arr = np.ascontiguousarray(np.concatenate(seg_cols, axis=1))
        dl_arr = np.ascontiguousarray(np.concatenate(dl_cols, axis=1))

        inv_pad = np.ones(NLOC_PAD, np.float32)
        inv_pad[0:NLOC] = inv[c * NLOC:(c + 1) * NLOC]
        invdeg_arr = np.ascontiguousarray(
            np.broadcast_to(inv_pad, (128, NLOC_PAD)))

        m = {
            "xT": xT_pad,
            "idx": idx_arr,
            "dl": dl_arr,
            "invdeg": invdeg_arr,
            "cnt": np.ascontiguousarray(cnts[c:c + 1]),
        }
        m.update(shared)
        in_maps.append(m)
    return in_maps, CH


_prog_cache = {}


def kernel(**inputs) -> np.ndarray:
    args = {k: np.asarray(v) for k, v in inputs.items()}
    in_maps, CH = prepare_inputs(
        args["x"], args["edge_index"], args["W_map"], args["b_map"],
        args["Wl1"], args["bl1"], args["Wr1"], args["Wl2"], args["bl2"],
        args["Wr2"], args["Wl3"], args["bl3"], args["Wr3"])

    if CH not in _prog_cache:
        _prog_cache[CH] = build_program(CH)
    nc = _prog_cache[CH]

    trace = os.environ.get("KERNEL_TRACE", "0") == "1"
    kw = {}
    if trace:
        import concourse.bass_utils as bu
        bu.upload_artifacts = lambda t: ""
        kw = dict(trace=True, tmpdir=os.environ.get(
            "KERNEL_TRACE_DIR", "/tmp/kernel_trace"))
    res = run_bass_kernel_spmd(nc, in_maps, list(range(NCORES)), **kw)
    if trace and res.exec_time_ns is not None:
        print(f"HW exec time: {res.exec_time_ns} ns")

    out = np.concatenate([res.results[c]["out"] for c in range(NCORES)], axis=0)
    return out.astype(np.float32)



# revision 2
# speedup vs baseline: 1.0254x; 1.0254x over previous
"""Trainium2 Bass kernel for nn_Net_83794811945603 (3-layer GraphSAGE, mean agg).

Computation (N=50000 nodes, E=800000 edges):
    h0 = x @ W_map + b_map                                  [N,128]
    h1 = relu(mean_agg(h0) @ Wl1 + bl1 + h0 @ Wr1)          [N,128]
    h2 = relu(mean_agg(h1) @ Wl2 + bl2 + h1 @ Wr2)          [N,256]
    out = log_softmax(mean_agg(h2) @ Wl3 + bl3 + h2 @ Wr3)  [N,40]
where mean_agg(h)[i] = mean over edges (s->i) of h[s].

Strategy (8 NeuronCores, SPMD):
  - Nodes sharded row-wise: core c owns nodes [c*6250, (c+1)*6250) and all
    edges whose dst lands there.  Weights replicated.
  - Per layer: full previous-layer features live in DRAM (AllGather output).
    Edge-gather h[src] rows via SWDGE dma_gather (int16 indices; the node
    table is split in two 25000-row halves so indices fit in int16).
  - Segment-sum by dst via TensorE one-hot matmuls: for each 128-edge chunk,
    aggT += G_chunk^T @ S_chunk where S[e, j] = (dst_local[e] == j), built on
    VectorE with a broadcast is_equal against an iota row.  Mean = aggT *
    inv_deg (free-axis broadcast multiply with a host-replicated table).
  - Layer 3 aggregates z3 = h2 @ Wl3 (40->pad 64 wide) instead of h2
    (256 wide): segment_mean commutes with the right-matmul, 4x less gather
    traffic.
  - All dense matmuls run in the transposed layout hT = [feat, node] so the
    contraction dim always sits on partitions; node-major copies (needed for
    the row-gather and the output) are produced with PE transposes.
"""

import os
import sys

sys.path.insert(0, "/opt/trn_rl_repo")

import numpy as np
import ml_dtypes

import concourse.bass as bass
import concourse.bacc as bacc
import concourse.tile as tile
import concourse.mybir as mybir
from concourse.bass_utils import run_bass_kernel_spmd
from concourse.library_config import mlp

F32 = mybir.dt.float32
BF16 = mybir.dt.bfloat16
I16 = mybir.dt.int16
I32 = mybir.dt.int32

N = 50000
E = 800000
F_IN = 500
F_IN_PAD = 512
D = 128            # ID_DIM == HID
D2 = 256           # 2*HID
NCLS = 40
NCLS_PAD = 128
NCORES = 8
NLOC = N // NCORES          # 6250
NT = (NLOC + 127) // 128    # 49 dst tiles per core
NLOC_PAD = NT * 128         # 6272
HALF = N // 2               # int16 index split
KSUP = 2                    # dst tiles per gather "supertile"
NSUP = (NT + KSUP - 1) // KSUP          # 13
SUP_TILES = [min(KSUP, NT - T * KSUP) for T in range(NSUP)]  # [4]*12 + [1]


def _ts(i, size=128):
    return slice(i * size, (i + 1) * size)


class _StopBuild(Exception):
    pass


def build_program(CH, build_stage=None):
    """Build the SPMD Bass program. CH = 128-edge chunks per (tile, half).
    build_stage (debug): 0 = stage0+AG only, 1 = +L1+AG, 2 = +L2+AG, 3 = full."""
    if build_stage is None:
        build_stage = int(os.environ.get("KERNEL_BUILD_STAGE", "3"))
    nc = bacc.Bacc("TRN2", target_bir_lowering=False, debug=False,
                   num_devices=NCORES, num_swdge_queues=4)

    SEG = CH * 128                        # index slots per (tile, half)
    NP = (SEG + 1023) // 1024             # gather pieces per (tile, half)
    # segment column offsets in dl (units of CH cols) / idx (units of CH*8)
    seg_tiles = []                        # (T, h) -> tiles in segment
    for T in range(NSUP):
        for h in (0, 1):
            seg_tiles.append(SUP_TILES[T])
    seg_dl_off = np.concatenate([[0], np.cumsum([t * CH for t in seg_tiles])])
    seg_ix_off = np.concatenate([[0], np.cumsum([t * CH * 8 for t in seg_tiles])])
    DL_COLS = int(seg_dl_off[-1])         # NT*2*CH
    IX_COLS = int(seg_ix_off[-1])         # NT*2*CH*8

    def seg_idx(T, h):
        return T * 2 + h

    # ---- I/O ----
    xT = nc.dram_tensor("xT", [F_IN_PAD, NLOC_PAD], BF16, kind="ExternalInput")
    idx_d = nc.dram_tensor("idx", [128, IX_COLS], I16, kind="ExternalInput")
    dl_d = nc.dram_tensor("dl", [128, DL_COLS], F32, kind="ExternalInput")
    invdeg_d = nc.dram_tensor("invdeg", [128, NLOC_PAD], F32, kind="ExternalInput")
    iota_d = nc.dram_tensor("iota", [128, 128], F32, kind="ExternalInput")
    ident_d = nc.dram_tensor("ident", [128, 128], BF16, kind="ExternalInput")
    ident32_d = nc.dram_tensor("ident32", [128, 128], F32, kind="ExternalInput")
    wmap_d = nc.dram_tensor("wmap", [128, F_IN_PAD], BF16, kind="ExternalInput")
    bmap_d = nc.dram_tensor("bmap", [128, 1], F32, kind="ExternalInput")
    wl1_d = nc.dram_tensor("wl1", [128, D], BF16, kind="ExternalInput")
    wr1_d = nc.dram_tensor("wr1", [128, D], BF16, kind="ExternalInput")
    bl1_d = nc.dram_tensor("bl1", [128, 1], F32, kind="ExternalInput")
    wl2_d = nc.dram_tensor("wl2", [128, D2], BF16, kind="ExternalInput")
    wr2_d = nc.dram_tensor("wr2", [128, D2], BF16, kind="ExternalInput")
    bl2_d = nc.dram_tensor("bl2", [128, 2], F32, kind="ExternalInput")
    wl3_d = nc.dram_tensor("wl3", [128, 2 * NCLS_PAD], BF16, kind="ExternalInput")
    wr3_d = nc.dram_tensor("wr3", [128, 2 * NCLS_PAD], BF16, kind="ExternalInput")
    bl3_d = nc.dram_tensor("bl3", [NCLS_PAD, 1], F32, kind="ExternalInput")
    cnt_d = nc.dram_tensor("cnt", [1, NT * 2 * NP], I32, kind="ExternalInput")
    out_d = nc.dram_tensor("out", [NLOC, NCLS], F32, kind="ExternalOutput")

    # internal DRAM
    h0loc = nc.dram_tensor("h0loc", [NLOC, D], BF16)
    h1loc = nc.dram_tensor("h1loc", [NLOC, D], BF16)
    z3loc = nc.dram_tensor("z3loc", [NLOC, NCLS_PAD], BF16)
    h0full = nc.dram_tensor("h0full", [N, D], BF16, addr_space="Shared")
    h1full = nc.dram_tensor("h1full", [N, D], BF16, addr_space="Shared")
    z3full = nc.dram_tensor("z3full", [N, NCLS_PAD], BF16, addr_space="Shared")

    groups = [list(range(NCORES))]

    from contextlib import ExitStack
    ctx_regs = ExitStack()
    with tile.TileContext(nc) as tc:
        with (
            tc.tile_pool(name="const", bufs=1) as cp,
            tc.tile_pool(name="hres", bufs=1) as hp,
            tc.tile_pool(name="gat", bufs=6) as gp,
            tc.tile_pool(name="sone", bufs=5) as sp,
            tc.tile_pool(name="work", bufs=3) as wp,
            tc.tile_pool(name="xin", bufs=4) as xp,
            tc.tile_pool(name="pa", bufs=2 * KSUP, space="PSUM") as pa,
            tc.tile_pool(name="po", bufs=2, space="PSUM") as po,
            tc.tile_pool(name="pt", bufs=2, space="PSUM") as pt,
        ):
            # ---- load constants ----
            idx_sb = cp.tile([128, IX_COLS], I16)
            dl_sb = cp.tile([128, DL_COLS], F32)
            invdeg = cp.tile([128, NLOC_PAD], F32)
            iota = cp.tile([128, 128], F32)
            ident = cp.tile([128, 128], BF16)
            ident32 = cp.tile([128, 128], F32)
            wmap = cp.tile([128, F_IN_PAD], BF16)
            bmap = cp.tile([128, 1], F32)
            wl1 = cp.tile([128, D], BF16)
            wr1 = cp.tile([128, D], BF16)
            bl1 = cp.tile([128, 1], F32)
            wl2 = cp.tile([128, D2], BF16)
            wr2 = cp.tile([128, D2], BF16)
            bl2 = cp.tile([128, 2], F32)
            wl3 = cp.tile([128, 2 * NCLS_PAD], BF16)
            wr3 = cp.tile([128, 2 * NCLS_PAD], BF16)
            bl3 = cp.tile([NCLS_PAD, 1], F32)
            cnt_sb = cp.tile([1, NT * 2 * NP], I32)
            for sb_t, dr in [(idx_sb, idx_d), (dl_sb, dl_d), (invdeg, invdeg_d),
                             (iota, iota_d), (ident, ident_d), (ident32, ident32_d), (wmap, wmap_d),
                             (bmap, bmap_d), (wl1, wl1_d), (wr1, wr1_d),
                             (bl1, bl1_d), (wl2, wl2_d), (wr2, wr2_d),
                             (bl2, bl2_d), (wl3, wl3_d), (wr3, wr3_d),
                             (bl3, bl3_d), (cnt_sb, cnt_d)]:
                nc.scalar.dma_start(out=sb_t[:], in_=dr[:])

            # persistent transposed activations (tags share slots over time)
            h0T = hp.tile([128, NLOC_PAD], BF16, tag="hA")
            h1T = hp.tile([128, NLOC_PAD], BF16, tag="hB")

            # ---- stage 0: h0T = W_map^T @ xT + b_map ----
            NH = NLOC_PAD // 2
            for half_n in (0, 1):
                slabs = [xp.tile([128, NH], BF16, tag="xs",
                                 name=f"xs{half_n}_{k}") for k in range(4)]
                for k in range(4):
                    nc.sync.dma_start(out=slabs[k][:],
                                      in_=xT[_ts(k), half_n * NH:(half_n + 1) * NH])
                nh_chunks = [(i * 512, min(512, NH - i * 512))
                             for i in range((NH + 511) // 512)]
                for n0, nw in nh_chunks:
                    ps = po.tile([128, 512], F32, tag="po")
                    for k in range(4):
                        nc.tensor.matmul(ps[:, 0:nw], wmap[:, _ts(k)],
                                         slabs[k][:, n0:n0 + nw],
                                         start=(k == 0), stop=(k == 3))
                    g0 = half_n * NH + n0
                    nc.vector.tensor_scalar(out=h0T[:, g0:g0 + nw],
                                            in0=ps[:, 0:nw],
                                            scalar1=bmap[:, 0:1], scalar2=None,
                                            op0=mybir.AluOpType.add)
            # node-major h0 -> DRAM
            for t in range(NT):
                ptr = pt.tile([128, 128], BF16, tag="pt")
                nc.tensor.transpose(ptr[:], h0T[:, _ts(t)], ident[:])
                nm = wp.tile([128, 128], BF16, tag="nm")
                nc.scalar.copy(nm[:], ptr[:])
                rows = min(128, NLOC - t * 128)
                nc.sync.dma_start(out=h0loc[t * 128: t * 128 + rows, :],
                                  in_=nm[0:rows, :])
            for q in (0, 1):
                nc.gpsimd.collective_compute(
                    "AllGather", mybir.AluOpType.bypass, replica_groups=groups,
                    ins=[h0loc[q * (NLOC // 2):(q + 1) * (NLOC // 2), :]],
                    outs=[h0full[q * HALF:(q + 1) * HALF, :]])

            # zero the gather-pool slots once: reg-skipped trailing slots
            # are never written, and 0 * S keeps matmul inputs finite
            for zi in range(6):
                gz = gp.tile([128, KSUP * CH, 128], BF16, tag="g",
                             name=f"gz{zi}")
                nc.vector.memset(gz[:], 0.0)

            # ---- generic edge aggregation ----
            self_queue = [0]
            SEGSZ = CH * 128
            pieces = []
            p0 = 0
            while p0 < SEGSZ:
                pieces.append((p0, min(1024, SEGSZ - p0)))
                p0 += 1024


            def aggregate(T, h, src_full, elem, pa_tiles, first, last):
                """Gather + one-hot matmuls for supertile T, half h.
                pa_tiles[ti]: per-dst-tile psum bank (one accumulation group
                per bank -- PSUM start zeroes a whole 2KB zero-region, so
                groups must not share banks)."""
                ntl = SUP_TILES[T]
                g = gp.tile([128, KSUP * CH, elem], BF16, tag="g")
                s = seg_idx(T, h)
                io0 = int(seg_ix_off[s])
                SEG = CH * 128
                # <=1024-idx single-packet gathers over the whole segment,
                # rotating queues for parallel Q7 descriptor generation
                nidx = ntl * SEG
                done = 0
                while done < nidx:
                    n = min(1024, nidx - done)
                    nc.gpsimd.dma_gather(
                        g[:, done // 128:(done + n) // 128, :],
                        src_full[h * HALF:(h + 1) * HALF, :],
                        idx_sb[:, io0 + done // 16: io0 + (done + n) // 16],
                        n, n, elem,
                        single_packet=False, queue_num=self_queue[0])
                    self_queue[0] = (self_queue[0] + 1) % 4
                    done += n
                d0 = int(seg_dl_off[s])
                sone = sp.tile([128, KSUP * CH, 128], BF16, tag="S")
                io_b = iota[:].rearrange("p (o j) -> p o j", o=1) \
                    .broadcast_to([128, ntl * CH, 128])
                dl_b = dl_sb[:, d0:d0 + ntl * CH] \
                    .rearrange("p (c o) -> p c o", o=1) \
                    .broadcast_to([128, ntl * CH, 128])
                nc.vector.tensor_tensor(out=sone[:, 0:ntl * CH, :], in0=io_b,
                                        in1=dl_b, op=mybir.AluOpType.is_equal)
                for ti in range(ntl):
                    for cc in range(CH):
                        j = ti * CH + cc
                        nc.tensor.matmul(
                            pa_tiles[ti][0:elem, :],
                            g[:, j, :], sone[:, j, :],
                            start=(first and cc == 0), stop=(last and cc == CH - 1))

            def layer_loop(src_full, elem, tile_body, mean_dt=BF16):
                """For each supertile: gather both halves, then per-tile body.
                tile_body(t, mean_sb) consumes the inv-deg-scaled aggT."""
                for T in range(NSUP):
                    ntl = SUP_TILES[T]
                    pa_tiles = [pa.tile([128, 128], F32, tag="pa",
                                        name=f"pa_{T}_{i}")
                                for i in range(ntl)]
                    aggregate(T, 0, src_full, elem, pa_tiles, True, False)
                    aggregate(T, 1, src_full, elem, pa_tiles, False, True)
                    for ti in range(ntl):
                        t = T * KSUP + ti
                        mean = wp.tile([128, 128], mean_dt, tag="mean")
                        nc.vector.tensor_tensor(
                            out=mean[0:elem, :], in0=pa_tiles[ti][0:elem, :],
                            in1=invdeg[0:elem, _ts(t)],
                            op=mybir.AluOpType.mult)
                        tile_body(t, mean)

            # ---- layer 1 ----
            def l1_body(t, mean):
                rows = min(128, NLOC - t * 128)
                p1 = po.tile([128, 128], F32, tag="po")
                nc.tensor.matmul(p1[:], wl1[:], mean[:], start=True, stop=False)
                nc.tensor.matmul(p1[:], wr1[:], h0T[:, _ts(t)],
                                 start=False, stop=True)
                nc.scalar.activation(out=h1T[:, _ts(t)], in_=p1[:],
                                     func=mybir.ActivationFunctionType.Relu,
                                     bias=bl1[:, 0:1], scale=1.0)
                ptr = pt.tile([128, 128], BF16, tag="pt")
                nc.tensor.transpose(ptr[:], h1T[:, _ts(t)], ident[:])
                nm = wp.tile([128, 128], BF16, tag="nm")
                nc.scalar.copy(nm[:], ptr[:])
                nc.sync.dma_start(out=h1loc[t * 128: t * 128 + rows, :],
                                  in_=nm[0:rows, :])

            if build_stage >= 1:
                layer_loop(h0full, D, l1_body)
                for q in (0, 1):
                    nc.gpsimd.collective_compute(
                        "AllGather", mybir.AluOpType.bypass,
                        replica_groups=groups,
                        ins=[h1loc[q * (NLOC // 2):(q + 1) * (NLOC // 2), :]],
                        outs=[h1full[q * HALF:(q + 1) * HALF, :]])

            # ---- layer 2 (+ z3 projection) ----
            h2T0 = hp.tile([128, NLOC_PAD], BF16, tag="hA")  # reuses h0T slot
            h2T1 = hp.tile([128, NLOC_PAD], BF16, tag="hC")

            def l2_body(t, mean):
                rows = min(128, NLOC - t * 128)
                for hh, (h2T_h, wcol) in enumerate(((h2T0, _ts(0)),
                                                    (h2T1, _ts(1)))):
                    p2 = po.tile([128, 128], F32, tag="po")
                    nc.tensor.matmul(p2[:], wl2[:, wcol], mean[:],
                                     start=True, stop=False)
                    nc.tensor.matmul(p2[:], wr2[:, wcol], h1T[:, _ts(t)],
                                     start=False, stop=True)
                    nc.scalar.activation(
                        out=h2T_h[:, _ts(t)], in_=p2[:],
                        func=mybir.ActivationFunctionType.Relu,
                        bias=bl2[:, hh:hh + 1], scale=1.0)
                # z3 = h2 @ Wl3 (transposed: z3T = Wl3^T @ h2T), 64-padded
                pz = po.tile([128, 128], F32, tag="po")
                nc.tensor.matmul(pz[0:NCLS_PAD, :], wl3[:, 0:NCLS_PAD],
                                 h2T0[:, _ts(t)], start=True, stop=False)
                nc.tensor.matmul(pz[0:NCLS_PAD, :],
                                 wl3[:, NCLS_PAD:2 * NCLS_PAD],
                                 h2T1[:, _ts(t)], start=False, stop=True)
                zt = wp.tile([NCLS_PAD, 128], BF16, tag="zt")
                nc.scalar.copy(zt[:], pz[0:NCLS_PAD, :])
                ptz = pt.tile([128, 128], BF16, tag="pt")
                nc.tensor.transpose(ptz[:, 0:NCLS_PAD], zt[:],
                                    ident[0:NCLS_PAD, 0:NCLS_PAD])
                nmz = wp.tile([128, 128], BF16, tag="nm")
                nc.scalar.copy(nmz[:, 0:NCLS_PAD], ptz[:, 0:NCLS_PAD])
                nc.sync.dma_start(out=z3loc[t * 128: t * 128 + rows, :],
                                  in_=nmz[0:rows, 0:NCLS_PAD])

            if build_stage >= 2:
                layer_loop(h1full, D, l2_body)
                for q in (0, 1):
                    nc.gpsimd.collective_compute(
                        "AllGather", mybir.AluOpType.bypass,
                        replica_groups=groups,
                        ins=[z3loc[q * (NLOC // 2):(q + 1) * (NLOC // 2), :]],
                        outs=[z3full[q * HALF:(q + 1) * HALF, :]])

            # ---- layer 3 + log_softmax ----
            def l3_body(t, mean):
                rows = min(128, NLOC - t * 128)
                p3 = po.tile([128, 128], F32, tag="po")
                nc.tensor.matmul(p3[0:NCLS_PAD, :], wr3[:, 0:NCLS_PAD],
                                 h2T0[:, _ts(t)], start=True, stop=False)
                nc.tensor.matmul(p3[0:NCLS_PAD, :],
                                 wr3[:, NCLS_PAD:2 * NCLS_PAD],
                                 h2T1[:, _ts(t)], start=False, stop=True)
                # mean (already inv-deg scaled) + wr3 term + bias
                W64 = 64
                comb = wp.tile([W64, 128], F32, tag="comb")
                nc.vector.tensor_tensor(out=comb[:], in0=mean[0:W64, :],
                                        in1=p3[0:W64, :],
                                        op=mybir.AluOpType.add)
                # bias add while still class-major (per-partition bias on ACT)
                comb2 = wp.tile([W64, 128], F32, tag="comb2")
                nc.scalar.activation(out=comb2[:], in_=comb[:],
                                     func=mybir.ActivationFunctionType.Identity,
                                     bias=bl3[0:W64, 0:1], scale=1.0)
                ptf = pt.tile([128, 128], F32, tag="pt")
                nc.tensor.transpose(ptf[:, 0:W64], comb2[:],
                                    ident32[0:W64, 0:W64])
                # log_softmax over the 40 valid class columns (ACT-heavy)
                xm = wp.tile([128, 1], F32, tag="xm")
                nc.vector.tensor_reduce(out=xm[:], in_=ptf[:, 0:NCLS],
                                        axis=mybir.AxisListType.X,
                                        op=mybir.AluOpType.max, negate=True)
                tt = wp.tile([128, NCLS], F32, tag="tt")
                nc.scalar.activation(out=tt[:], in_=ptf[:, 0:NCLS],
                                     func=mybir.ActivationFunctionType.Identity,
                                     bias=xm[:, 0:1], scale=1.0)
                ex = wp.tile([128, NCLS], F32, tag="ex")
                ssum = wp.tile([128, 1], F32, tag="ssum")
                nc.scalar.activation(out=ex[:], in_=tt[:],
                                     func=mybir.ActivationFunctionType.Exp,
                                     accum_out=ssum[:])
                lse = wp.tile([128, 1], F32, tag="lse")
                nc.scalar.activation(out=lse[:], in_=ssum[:],
                                     func=mybir.ActivationFunctionType.Ln)
                lsn = wp.tile([128, 1], F32, tag="lsn")
                nc.scalar.mul(lsn[:], lse[:], -1.0)
                fin = wp.tile([128, NCLS], F32, tag="fin")
                nc.scalar.activation(out=fin[:], in_=tt[:],
                                     func=mybir.ActivationFunctionType.Identity,
                                     bias=lsn[:, 0:1], scale=1.0)
                nc.sync.dma_start(out=out_d[t * 128: t * 128 + rows, :],
                                  in_=fin[0:rows, :])

            if build_stage >= 3:
                layer_loop(z3full, NCLS_PAD, l3_body, mean_dt=F32)

    nc.compile()
    return nc


# ---------------- host side ----------------

def _pack_idx_segment(vals: np.ndarray) -> np.ndarray:
    """[L] int16 -> [128, L//16]: slot i -> [i % 16, i // 16], x8 replicated."""
    L = vals.shape[0]
    arr = vals.reshape(L // 16, 16).T  # [16, L//16]
    return np.tile(arr, (8, 1))


def prepare_inputs(x, edge_index, W_map, b_map, Wl1, bl1, Wr1, Wl2, bl2, Wr2,
                   Wl3, bl3, Wr3):
    src = np.asarray(edge_index[0], dtype=np.int64)
    dst = np.asarray(edge_index[1], dtype=np.int64)

    core = dst // NLOC
    local = dst - core * NLOC
    t_loc = local >> 7
    dloc = local & 127
    # chunked-AG table layout: chunk q holds all cores' local rows
    # [q*3125, (q+1)*3125): position = c_src*3125 + (r - q*3125)
    c_src = src // NLOC
    r_src = src - c_src * NLOC
    half = (r_src >= NLOC // 2).astype(np.int64)
    idx16 = (c_src * (NLOC // 2) + (r_src - half * (NLOC // 2))).astype(np.int16)

    T_sup = t_loc // KSUP
    ti = t_loc - T_sup * KSUP

    # fine group for slot assignment: (core, supertile, half, tile_in_sup)
    fine = ((core * NSUP + T_sup) * 2 + half) * KSUP + ti
    NFINE = NCORES * NSUP * 2 * KSUP
    counts = np.bincount(fine, minlength=NFINE)
    CH = int(np.ceil(counts.max() / 128))
    SEG = CH * 128

    order = np.argsort(fine, kind="stable")
    fine_s = fine[order]
    offs = np.concatenate([[0], np.cumsum(counts)])
    pos = np.arange(E) - np.repeat(offs[:-1], counts)

    # flat slot layout per core: segments (T, h) with ti-major inner blocks,
    # but careful: supertile T has SUP_TILES[T] tiles; fine group (T, h, ti)
    # occupies slots [seg_base(T,h) + ti*SEG, ...+counts).
    seg_tiles = []
    for T in range(NSUP):
        for h in (0, 1):
            seg_tiles.append(SUP_TILES[T])
    seg_slot_off = np.concatenate([[0], np.cumsum([t * SEG for t in seg_tiles])])
    SLTOT = int(seg_slot_off[-1])  # slots per core

    # fine group -> global slot base
    fine_base = np.zeros(NFINE, dtype=np.int64)
    for c in range(NCORES):
        for T in range(NSUP):
            for h in (0, 1):
                s = T * 2 + h
                for ti_ in range(SUP_TILES[T]):
                    f = ((c * NSUP + T) * 2 + h) * KSUP + ti_
                    fine_base[f] = c * SLTOT + seg_slot_off[s] + ti_ * SEG

    slot = fine_base[fine_s] + pos  # global slot per sorted edge

    big_idx = np.full(NCORES * SLTOT, 0, dtype=np.int16)
    big_dl = np.full(NCORES * SLTOT, 999.0, dtype=np.float32)
    big_idx[slot] = idx16[order]
    big_dl[slot] = dloc[order].astype(np.float32)
    big_idx = big_idx.reshape(NCORES, SLTOT)
    big_dl = big_dl.reshape(NCORES, SLTOT)

    # per-(core, tile, half, piece) valid counts; ensure >=1 valid per piece
    SEGSZ = CH * 128
    pieces = []
    p0 = 0
    while p0 < SEGSZ:
        pieces.append((p0, min(1024, SEGSZ - p0)))
        p0 += 1024
    NP = len(pieces)
    cnts = np.zeros((NCORES, NT * 2 * NP), dtype=np.int32)
    fine_v = counts  # per fine group (c,T,h,ti)
    for c in range(NCORES):
        for T in range(NSUP):
            for h in (0, 1):
                sg = T * 2 + h
                for ti_ in range(SUP_TILES[T]):
                    f = ((c * NSUP + T) * 2 + h) * KSUP + ti_
                    v = int(fine_v[f])
                    t_glob = T * KSUP + ti_
                    base = c * SLTOT + int(seg_slot_off[sg]) + ti_ * SEGSZ
                    for pi, (q0, plen) in enumerate(pieces):
                        cnts[c, (t_glob * 2 + h) * NP + pi] = plen

    # degrees
    cnt = np.bincount(dst, minlength=N).astype(np.float32)
    inv = 1.0 / np.maximum(cnt, 1.0)

    # weights (shared)
    BF = ml_dtypes.bfloat16
    Wmap_pad = np.zeros((F_IN_PAD, 128), np.float32)
    Wmap_pad[0:F_IN] = W_map
    wmap_kt = np.concatenate([Wmap_pad[_ts(k)] for k in range(4)], axis=1)
    Wl3_pad = np.zeros((D2, NCLS_PAD), np.float32)
    Wl3_pad[:, 0:NCLS] = Wl3
    wl3_kt = np.concatenate([Wl3_pad[_ts(k)] for k in range(2)], axis=1)
    Wr3_pad = np.zeros((D2, NCLS_PAD), np.float32)
    Wr3_pad[:, 0:NCLS] = Wr3
    wr3_kt = np.concatenate([Wr3_pad[_ts(k)] for k in range(2)], axis=1)
    bl3_pad = np.zeros((NCLS_PAD, 1), np.float32)
    bl3_pad[0:NCLS, 0] = bl3

    shared = {
        "iota": np.ascontiguousarray(
            np.tile(np.arange(128, dtype=np.float32), (128, 1))),
        "ident": np.eye(128, dtype=np.float32).astype(BF),
        "ident32": np.eye(128, dtype=np.float32),
        "wmap": np.ascontiguousarray(wmap_kt).astype(BF),
        "bmap": np.ascontiguousarray(b_map.reshape(128, 1)),
        "wl1": np.ascontiguousarray(Wl1).astype(BF),
        "wr1": np.ascontiguousarray(Wr1).astype(BF),
        "bl1": np.ascontiguousarray(bl1.reshape(128, 1)),
        "wl2": np.ascontiguousarray(Wl2).astype(BF),
        "wr2": np.ascontiguousarray(Wr2).astype(BF),
        "bl2": np.ascontiguousarray(bl2.reshape(2, 128).T),
        "wl3": np.ascontiguousarray(wl3_kt).astype(BF),
        "wr3": np.ascontiguousarray(wr3_kt).astype(BF),
        "bl3": bl3_pad,
    }

    in_maps = []
    for c in range(NCORES):
        xT_pad = np.zeros((F_IN_PAD, NLOC_PAD), np.float32)
        xT_pad[0:F_IN, 0:NLOC] = x[c * NLOC:(c + 1) * NLOC].T
        xT_pad = xT_pad.astype(ml_dtypes.bfloat16)

        # idx layout: per (T,h) segment packed independently, concat cols
        seg_cols = []
        dl_cols = []
        for T in range(NSUP):
            for h in (0, 1):
                s = T * 2 + h
                a, b = int(seg_slot_off[s]), int(seg_slot_off[s + 1])
                vals = big_idx[c, a:b]
                seg_cols.append(_pack_idx_segment(vals))
                dls = big_dl[c, a:b].reshape(-1, 128).T  # [128, tiles*CH]
                dl_cols.append(dls)
        idx_arr = np.ascontiguousarray(np.concatenate(seg_cols, axis=1))
        dl_arr = np.ascontiguousarray(np.concatenate(dl_cols, axis=1))

        inv_pad = np.ones(NLOC_PAD, np.float32)
        inv_pad[0:NLOC] = inv[c * NLOC:(c + 1) * NLOC]
        invdeg_arr = np.ascontiguousarray(
            np.broadcast_to(inv_pad, (128, NLOC_PAD)))

        m = {
            "xT": xT_pad,
            "idx": idx_arr,
            "dl": dl_arr,
            "invdeg": invdeg_arr,
            "cnt": np.ascontiguousarray(cnts[c:c + 1]),
        }
        m.update(shared)
        in_maps.append(m)
    return in_maps, CH


_prog_cache = {}


def kernel(**inputs) -> np.ndarray:
    args = {k: np.asarray(v) for k, v in inputs.items()}
    in_maps, CH = prepare_inputs(
        args["x"], args["edge_index"], args["W_map"], args["b_map"],
        args["Wl1"], args["bl1"], args["Wr1"], args["Wl2"], args["bl2"],
        args["Wr2"], args["Wl3"], args["bl3"], args["Wr3"])

    if CH not in _prog_cache:
        _prog_cache[CH] = build_program(CH)
    nc = _prog_cache[CH]

    trace = os.environ.get("KERNEL_TRACE", "0") == "1"
    kw = {}
    if trace:
        import concourse.bass_utils as bu
        bu.upload_artifacts = lambda t: ""
        kw = dict(trace=True, tmpdir=os.environ.get(
            "KERNEL_TRACE_DIR", "/tmp/kernel_trace"))
    res = run_bass_kernel_spmd(nc, in_maps, list(range(NCORES)), **kw)
    if trace and res.exec_time_ns is not None:
        print(f"HW exec time: {res.exec_time_ns} ns")

    out = np.concatenate([res.results[c]["out"] for c in range(NCORES)], axis=0)
    return out.astype(np.float32)



# revision 4
# speedup vs baseline: 1.0773x; 1.0506x over previous
"""Trainium2 Bass kernel for nn_Net_83794811945603 (3-layer GraphSAGE, mean agg).

Computation (N=50000 nodes, E=800000 edges):
    h0 = x @ W_map + b_map                                  [N,128]
    h1 = relu(mean_agg(h0) @ Wl1 + bl1 + h0 @ Wr1)          [N,128]
    h2 = relu(mean_agg(h1) @ Wl2 + bl2 + h1 @ Wr2)          [N,256]
    out = log_softmax(mean_agg(h2) @ Wl3 + bl3 + h2 @ Wr3)  [N,40]
where mean_agg(h)[i] = mean over edges (s->i) of h[s].

Strategy (8 NeuronCores, SPMD):
  - Nodes sharded row-wise: core c owns nodes [c*6250, (c+1)*6250) and all
    edges whose dst lands there.  Weights replicated.
  - Per layer: full previous-layer features live in DRAM (AllGather output).
    Edge-gather h[src] rows via SWDGE dma_gather (int16 indices; the node
    table is split in two 25000-row halves so indices fit in int16).
    One gather call per (supertile, half) segment (up to 4608 indices) with
    single_packet=False so the drain stripes across all 16 SDMA engines.
  - Segment-sum by dst via TensorE one-hot matmuls: for each 128-edge chunk,
    aggT += G_chunk^T @ S_chunk where S[e, j] = (dst_local[e] == j), built on
    VectorE with a broadcast is_equal against an iota row (bf16 operands).
    Mean = aggT * inv_deg (free-axis broadcast multiply).
  - Layer 3 aggregates z3 = h2 @ Wl3 (40->pad wide) instead of h2
    (256 wide): segment_mean commutes with the right-matmul, 4x less gather
    traffic.
  - AllGathers are split in 2 halves per layer and triggered mid-layer as
    soon as their input rows are written, overlapping collective latency
    with the back half of each layer's aggregation.
"""

import os
import sys

sys.path.insert(0, "/opt/trn_rl_repo")

import numpy as np
import ml_dtypes

import concourse.bass as bass
import concourse.bacc as bacc
import concourse.tile as tile
import concourse.mybir as mybir
from concourse.bass_utils import run_bass_kernel_spmd

F32 = mybir.dt.float32
BF16 = mybir.dt.bfloat16
I16 = mybir.dt.int16

N = 50000
E = 800000
F_IN = 500
F_IN_PAD = 512
D = 128            # ID_DIM == HID
D2 = 256           # 2*HID
NCLS = 40
NCLS_PAD = 128
NCORES = 8
NLOC = N // NCORES          # 6250
NT = (NLOC + 127) // 128    # 49 dst tiles per core
NLOC_PAD = NT * 128         # 6272
HALF = N // 2               # int16 index split
KSUP = 4                    # dst tiles per gather "supertile"
NSUP = (NT + KSUP - 1) // KSUP          # 13
SUP_TILES = [min(KSUP, NT - T * KSUP) for T in range(NSUP)]  # [4]*12 + [1]


def _ts(i, size=128):
    return slice(i * size, (i + 1) * size)


def build_program(CH, build_stage=None):
    """Build the SPMD Bass program. CH = 128-edge chunks per (tile, half)."""
    if build_stage is None:
        build_stage = int(os.environ.get("KERNEL_BUILD_STAGE", "3"))
    nc = bacc.Bacc("TRN2", target_bir_lowering=False, debug=False,
                   num_devices=NCORES, num_swdge_queues=4)

    SEG = CH * 128                        # index slots per (tile, half)
    # segment column offsets in dl (units of CH cols) / idx (units of CH*8)
    seg_tiles = []                        # (T, h) -> tiles in segment
    for T in range(NSUP):
        for h in (0, 1):
            seg_tiles.append(SUP_TILES[T])
    seg_dl_off = np.concatenate([[0], np.cumsum([t * CH for t in seg_tiles])])
    seg_ix_off = np.concatenate([[0], np.cumsum([t * CH * 8 for t in seg_tiles])])
    DL_COLS = int(seg_dl_off[-1])         # NT*2*CH
    IX_COLS = int(seg_ix_off[-1])         # NT*2*CH*8

    def seg_idx(T, h):
        return T * 2 + h

    # ---- I/O ----
    xT = nc.dram_tensor("xT", [F_IN_PAD, NLOC_PAD], BF16, kind="ExternalInput")
    idx_d = nc.dram_tensor("idx", [128, IX_COLS], I16, kind="ExternalInput")
    dl_d = nc.dram_tensor("dl", [128, DL_COLS], BF16, kind="ExternalInput")
    invdeg_d = nc.dram_tensor("invdeg", [128, NLOC_PAD], F32, kind="ExternalInput")
    iota_d = nc.dram_tensor("iota", [128, 128], BF16, kind="ExternalInput")
    ident_d = nc.dram_tensor("ident", [128, 128], BF16, kind="ExternalInput")
    ident32_d = nc.dram_tensor("ident32", [128, 128], F32, kind="ExternalInput")
    wmap_d = nc.dram_tensor("wmap", [128, F_IN_PAD], BF16, kind="ExternalInput")
    bmap_d = nc.dram_tensor("bmap", [128, 1], F32, kind="ExternalInput")
    wl1_d = nc.dram_tensor("wl1", [128, D], BF16, kind="ExternalInput")
    wr1_d = nc.dram_tensor("wr1", [128, D], BF16, kind="ExternalInput")
    bl1_d = nc.dram_tensor("bl1", [128, 1], F32, kind="ExternalInput")
    wl2_d = nc.dram_tensor("wl2", [128, D2], BF16, kind="ExternalInput")
    wr2_d = nc.dram_tensor("wr2", [128, D2], BF16, kind="ExternalInput")
    bl2_d = nc.dram_tensor("bl2", [128, 2], F32, kind="ExternalInput")
    wl3_d = nc.dram_tensor("wl3", [128, 2 * NCLS_PAD], BF16, kind="ExternalInput")
    wr3_d = nc.dram_tensor("wr3", [128, 2 * NCLS_PAD], BF16, kind="ExternalInput")
    bl3_d = nc.dram_tensor("bl3", [NCLS_PAD, 1], F32, kind="ExternalInput")
    out_d = nc.dram_tensor("out", [NLOC, NCLS], F32, kind="ExternalOutput")

    # internal DRAM
    h0loc = nc.dram_tensor("h0loc", [NLOC, D], BF16)
    h1loc = nc.dram_tensor("h1loc", [NLOC, D], BF16)
    z3loc = nc.dram_tensor("z3loc", [NLOC, NCLS_PAD], BF16)
    h0full = nc.dram_tensor("h0full", [N, D], BF16, addr_space="Shared")
    h1full = nc.dram_tensor("h1full", [N, D], BF16, addr_space="Shared")
    z3full = nc.dram_tensor("z3full", [N, NCLS_PAD], BF16, addr_space="Shared")

    groups = [list(range(NCORES))]

    with tile.TileContext(nc) as tc:
        with (
            tc.tile_pool(name="const", bufs=1) as cp,
            tc.tile_pool(name="hres", bufs=1) as hp,
            tc.tile_pool(name="gat", bufs=3) as gp,
            tc.tile_pool(name="sone", bufs=3) as sp,
            tc.tile_pool(name="work", bufs=3) as wp,
            tc.tile_pool(name="xin", bufs=4) as xp,
            tc.tile_pool(name="pa", bufs=KSUP, space="PSUM") as pa,
            tc.tile_pool(name="po", bufs=2, space="PSUM") as po,
            tc.tile_pool(name="pt", bufs=2, space="PSUM") as pt,
        ):
            # ---- load constants ----
            idx_sb = cp.tile([128, IX_COLS], I16)
            dl_sb = cp.tile([128, DL_COLS], BF16)
            invdeg = cp.tile([128, NLOC_PAD], F32)
            iota = cp.tile([128, 128], BF16)
            ident = cp.tile([128, 128], BF16)
            ident32 = cp.tile([128, 128], F32)
            wmap = cp.tile([128, F_IN_PAD], BF16)
            bmap = cp.tile([128, 1], F32)
            wl1 = cp.tile([128, D], BF16)
            wr1 = cp.tile([128, D], BF16)
            bl1 = cp.tile([128, 1], F32)
            wl2 = cp.tile([128, D2], BF16)
            wr2 = cp.tile([128, D2], BF16)
            bl2 = cp.tile([128, 2], F32)
            wl3 = cp.tile([128, 2 * NCLS_PAD], BF16)
            wr3 = cp.tile([128, 2 * NCLS_PAD], BF16)
            bl3 = cp.tile([NCLS_PAD, 1], F32)
            for sb_t, dr in [(idx_sb, idx_d), (dl_sb, dl_d), (invdeg, invdeg_d),
                             (iota, iota_d), (ident, ident_d),
                             (ident32, ident32_d), (wmap, wmap_d),
                             (bmap, bmap_d), (wl1, wl1_d), (wr1, wr1_d),
                             (bl1, bl1_d), (wl2, wl2_d), (wr2, wr2_d),
                             (bl2, bl2_d), (wl3, wl3_d), (wr3, wr3_d),
                             (bl3, bl3_d)]:
                nc.scalar.dma_start(out=sb_t[:], in_=dr[:])

            # persistent transposed activations (tags share slots over time)
            h0T = hp.tile([128, NLOC_PAD], BF16, tag="hA")
            h1T = hp.tile([128, NLOC_PAD], BF16, tag="hB")

            # ---- stage 0: h0T = W_map^T @ xT + b_map ----
            NH = NLOC_PAD // 2
            for half_n in (0, 1):
                slabs = [xp.tile([128, NH], BF16, tag="xs",
                                 name=f"xs{half_n}_{k}") for k in range(4)]
                for k in range(4):
                    nc.sync.dma_start(out=slabs[k][:],
                                      in_=xT[_ts(k), half_n * NH:(half_n + 1) * NH])
                nh_chunks = [(i * 512, min(512, NH - i * 512))
                             for i in range((NH + 511) // 512)]
                for n0, nw in nh_chunks:
                    ps = po.tile([128, 512], F32, tag="po")
                    for k in range(4):
                        nc.tensor.matmul(ps[:, 0:nw], wmap[:, _ts(k)],
                                         slabs[k][:, n0:n0 + nw],
                                         start=(k == 0), stop=(k == 3))
                    g0 = half_n * NH + n0
                    nc.vector.tensor_scalar(out=h0T[:, g0:g0 + nw],
                                            in0=ps[:, 0:nw],
                                            scalar1=bmap[:, 0:1], scalar2=None,
                                            op0=mybir.AluOpType.add)
            # node-major h0 -> DRAM
            for t in range(NT):
                ptr = pt.tile([128, 128], BF16, tag="pt")
                nc.tensor.transpose(ptr[:], h0T[:, _ts(t)], ident[:])
                nm = wp.tile([128, 128], BF16, tag="nm")
                nc.scalar.copy(nm[:], ptr[:])
                rows = min(128, NLOC - t * 128)
                nc.sync.dma_start(out=h0loc[t * 128: t * 128 + rows, :],
                                  in_=nm[0:rows, :])
            for q in (0, 1):
                nc.gpsimd.collective_compute(
                    "AllGather", mybir.AluOpType.bypass, replica_groups=groups,
                    ins=[h0loc[q * (NLOC // 2):(q + 1) * (NLOC // 2), :]],
                    outs=[h0full[q * HALF:(q + 1) * HALF, :]])

            # zero the gather-pool slots once: the last supertile only
            # writes 1 of KSUP tile groups, and 0 * S keeps matmuls finite
            for zi in range(3):
                gz = gp.tile([128, KSUP * CH, 128], BF16, tag="g",
                             name=f"gz{zi}")
                nc.vector.memset(gz[:], 0.0)

            # ---- generic edge aggregation ----
            self_queue = [0]

            def aggregate(T, h, src_full, elem, pa_tiles, first, last):
                """Gather + one-hot matmuls for supertile T, half h.
                One dma_gather covers the whole (T, h) segment; the drain
                stripes across all 16 SDMA engines (single_packet=False).
                pa_tiles[ti]: per-dst-tile psum bank (one accumulation group
                per bank -- PSUM start zeroes a whole 2KB zero-region, so
                groups must not share banks)."""
                ntl = SUP_TILES[T]
                g = gp.tile([128, KSUP * CH, elem], BF16, tag="g")
                s = seg_idx(T, h)
                io0 = int(seg_ix_off[s])
                nidx = ntl * CH * 128
                done = 0
                while done < nidx:
                    n = min(1024, nidx - done)
                    nc.gpsimd.dma_gather(
                        g[:, done // 128:(done + n) // 128, :],
                        src_full[h * HALF:(h + 1) * HALF, :],
                        idx_sb[:, io0 + done // 16: io0 + (done + n) // 16],
                        n, n, elem,
                        single_packet=True, queue_num=self_queue[0])
                    self_queue[0] = (self_queue[0] + 1) % 4
                    done += n
                d0 = int(seg_dl_off[s])
                sone = sp.tile([128, KSUP * CH, 128], BF16, tag="S")
                io_b = iota[:].rearrange("p (o j) -> p o j", o=1) \
                    .broadcast_to([128, ntl * CH, 128])
                dl_b = dl_sb[:, d0:d0 + ntl * CH] \
                    .rearrange("p (c o) -> p c o", o=1) \
                    .broadcast_to([128, ntl * CH, 128])
                nc.vector.tensor_tensor(out=sone[:, 0:ntl * CH, :], in0=io_b,
                                        in1=dl_b, op=mybir.AluOpType.is_equal)
                for ti in range(ntl):
                    for cc in range(CH):
                        j = ti * CH + cc
                        nc.tensor.matmul(
                            pa_tiles[ti][0:elem, :],
                            g[:, j, :], sone[:, j, :],
                            start=(first and cc == 0),
                            stop=(last and cc == CH - 1))

            def layer_loop(src_full, elem, tile_body, mean_dt=BF16,
                           post_supertile=None):
                """For each supertile: gather both halves, then per-tile body.
                tile_body(t, mean_sb) consumes the inv-deg-scaled aggT.
                post_supertile: optional {T: fn} hooks (AllGather triggers)."""
                for T in range(NSUP):
                    ntl = SUP_TILES[T]
                    pa_tiles = [pa.tile([128, 128], F32, tag="pa",
                                        name=f"pa_{T}_{i}")
                                for i in range(ntl)]
                    aggregate(T, 0, src_full, elem, pa_tiles, True, False)
                    aggregate(T, 1, src_full, elem, pa_tiles, False, True)
                    for ti in range(ntl):
                        t = T * KSUP + ti
                        mean = wp.tile([128, 128], mean_dt, tag="mean")
                        nc.vector.tensor_tensor(
                            out=mean[0:elem, :], in0=pa_tiles[ti][0:elem, :],
                            in1=invdeg[0:elem, _ts(t)],
                            op=mybir.AluOpType.mult)
                        tile_body(t, mean)
                    if post_supertile and T in post_supertile:
                        post_supertile[T]()

            # ---- layer 1 ----
            def l1_body(t, mean):
                rows = min(128, NLOC - t * 128)
                p1 = po.tile([128, 128], F32, tag="po")
                nc.tensor.matmul(p1[:], wl1[:], mean[:], start=True, stop=False)
                nc.tensor.matmul(p1[:], wr1[:], h0T[:, _ts(t)],
                                 start=False, stop=True)
                nc.scalar.activation(out=h1T[:, _ts(t)], in_=p1[:],
                                     func=mybir.ActivationFunctionType.Relu,
                                     bias=bl1[:, 0:1], scale=1.0)
                ptr = pt.tile([128, 128], BF16, tag="pt")
                nc.tensor.transpose(ptr[:], h1T[:, _ts(t)], ident[:])
                nm = wp.tile([128, 128], BF16, tag="nm")
                nc.scalar.copy(nm[:], ptr[:])
                nc.sync.dma_start(out=h1loc[t * 128: t * 128 + rows, :],
                                  in_=nm[0:rows, :])

            def ag_trigger(loc, full, q):
                def fn():
                    nc.gpsimd.collective_compute(
                        "AllGather", mybir.AluOpType.bypass,
                        replica_groups=groups,
                        ins=[loc[q * (NLOC // 2):(q + 1) * (NLOC // 2), :]],
                        outs=[full[q * HALF:(q + 1) * HALF, :]])
                return fn

            # supertile 6 covers tiles 24-27; rows [0, 3125) are tiles 0-24
            if build_stage >= 1:
                layer_loop(h0full, D, l1_body,
                           post_supertile={6: ag_trigger(h1loc, h1full, 0),
                                           NSUP - 1: ag_trigger(h1loc, h1full, 1)})

            # ---- layer 2 (+ z3 projection) ----
            h2T0 = hp.tile([128, NLOC_PAD], BF16, tag="hA")  # reuses h0T slot
            h2T1 = hp.tile([128, NLOC_PAD], BF16, tag="hC")

            def l2_body(t, mean):
                rows = min(128, NLOC - t * 128)
                for hh, (h2T_h, wcol) in enumerate(((h2T0, _ts(0)),
                                                    (h2T1, _ts(1)))):
                    p2 = po.tile([128, 128], F32, tag="po")
                    nc.tensor.matmul(p2[:], wl2[:, wcol], mean[:],
                                     start=True, stop=False)
                    nc.tensor.matmul(p2[:], wr2[:, wcol], h1T[:, _ts(t)],
                                     start=False, stop=True)
                    nc.scalar.activation(
                        out=h2T_h[:, _ts(t)], in_=p2[:],
                        func=mybir.ActivationFunctionType.Relu,
                        bias=bl2[:, hh:hh + 1], scale=1.0)
                # z3 = h2 @ Wl3 (transposed: z3T = Wl3^T @ h2T), 64-padded
                pz = po.tile([128, 128], F32, tag="po")
                nc.tensor.matmul(pz[0:NCLS_PAD, :], wl3[:, 0:NCLS_PAD],
                                 h2T0[:, _ts(t)], start=True, stop=False)
                nc.tensor.matmul(pz[0:NCLS_PAD, :],
                                 wl3[:, NCLS_PAD:2 * NCLS_PAD],
                                 h2T1[:, _ts(t)], start=False, stop=True)
                zt = wp.tile([NCLS_PAD, 128], BF16, tag="zt")
                nc.scalar.copy(zt[:], pz[0:NCLS_PAD, :])
                ptz = pt.tile([128, 128], BF16, tag="pt")
                nc.tensor.transpose(ptz[:, 0:NCLS_PAD], zt[:],
                                    ident[0:NCLS_PAD, 0:NCLS_PAD])
                nmz = wp.tile([128, 128], BF16, tag="nm")
                nc.scalar.copy(nmz[:, 0:NCLS_PAD], ptz[:, 0:NCLS_PAD])
                nc.sync.dma_start(out=z3loc[t * 128: t * 128 + rows, :],
                                  in_=nmz[0:rows, 0:NCLS_PAD])

            if build_stage >= 2:
                layer_loop(h1full, D, l2_body,
                           post_supertile={6: ag_trigger(z3loc, z3full, 0),
                                           NSUP - 1: ag_trigger(z3loc, z3full, 1)})

            # ---- layer 3 + log_softmax ----
            def l3_body(t, mean):
                rows = min(128, NLOC - t * 128)
                p3 = po.tile([128, 128], F32, tag="po")
                nc.tensor.matmul(p3[0:NCLS_PAD, :], wr3[:, 0:NCLS_PAD],
                                 h2T0[:, _ts(t)], start=True, stop=False)
                nc.tensor.matmul(p3[0:NCLS_PAD, :],
                                 wr3[:, NCLS_PAD:2 * NCLS_PAD],
                                 h2T1[:, _ts(t)], start=False, stop=True)
                # mean (already inv-deg scaled) + wr3 term + bias
                W64 = 64
                comb = wp.tile([W64, 128], F32, tag="comb")
                nc.vector.tensor_tensor(out=comb[:], in0=mean[0:W64, :],
                                        in1=p3[0:W64, :],
                                        op=mybir.AluOpType.add)
                # bias add while still class-major (per-partition bias on ACT)
                comb2 = wp.tile([W64, 128], F32, tag="comb2")
                nc.scalar.activation(out=comb2[:], in_=comb[:],
                                     func=mybir.ActivationFunctionType.Identity,
                                     bias=bl3[0:W64, 0:1], scale=1.0)
                ptf = pt.tile([128, 128], F32, tag="pt")
                nc.tensor.transpose(ptf[:, 0:W64], comb2[:],
                                    ident32[0:W64, 0:W64])
                # log_softmax over the 40 valid class columns (ACT-heavy)
                xm = wp.tile([128, 1], F32, tag="xm")
                nc.vector.tensor_reduce(out=xm[:], in_=ptf[:, 0:NCLS],
                                        axis=mybir.AxisListType.X,
                                        op=mybir.AluOpType.max, negate=True)
                tt = wp.tile([128, NCLS], F32, tag="tt")
                nc.scalar.activation(out=tt[:], in_=ptf[:, 0:NCLS],
                                     func=mybir.ActivationFunctionType.Identity,
                                     bias=xm[:, 0:1], scale=1.0)
                ex = wp.tile([128, NCLS], F32, tag="ex")
                ssum = wp.tile([128, 1], F32, tag="ssum")
                nc.scalar.activation(out=ex[:], in_=tt[:],
                                     func=mybir.ActivationFunctionType.Exp,
                                     accum_out=ssum[:])
                lse = wp.tile([128, 1], F32, tag="lse")
                nc.scalar.activation(out=lse[:], in_=ssum[:],
                                     func=mybir.ActivationFunctionType.Ln)
                lsn = wp.tile([128, 1], F32, tag="lsn")
                nc.scalar.mul(lsn[:], lse[:], -1.0)
                fin = wp.tile([128, NCLS], F32, tag="fin")
                nc.scalar.activation(out=fin[:], in_=tt[:],
                                     func=mybir.ActivationFunctionType.Identity,
                                     bias=lsn[:, 0:1], scale=1.0)
                nc.sync.dma_start(out=out_d[t * 128: t * 128 + rows, :],
                                  in_=fin[0:rows, :])

            if build_stage >= 3:
                layer_loop(z3full, NCLS_PAD, l3_body, mean_dt=F32)

    nc.compile()
    return nc


# ---------------- host side ----------------

def _pack_idx_segment(vals: np.ndarray) -> np.ndarray:
    """[L] int16 -> [128, L//16]: slot i -> [i % 16, i // 16], x8 replicated."""
    L = vals.shape[0]
    arr = vals.reshape(L // 16, 16).T  # [16, L//16]
    return np.tile(arr, (8, 1))


def prepare_inputs(x, edge_index, W_map, b_map, Wl1, bl1, Wr1, Wl2, bl2, Wr2,
                   Wl3, bl3, Wr3):
    src = np.asarray(edge_index[0], dtype=np.int64)
    dst = np.asarray(edge_index[1], dtype=np.int64)

    core = dst // NLOC
    local = dst - core * NLOC
    t_loc = local >> 7
    dloc = local & 127
    # chunked-AG table layout: chunk q holds all cores' local rows
    # [q*3125, (q+1)*3125): position = c_src*3125 + (r - q*3125)
    c_src = src // NLOC
    r_src = src - c_src * NLOC
    half = (r_src >= NLOC // 2).astype(np.int64)
    idx16 = (c_src * (NLOC // 2) + (r_src - half * (NLOC // 2))).astype(np.int16)

    T_sup = t_loc // KSUP
    ti = t_loc - T_sup * KSUP

    # fine group for slot assignment: (core, supertile, half, tile_in_sup)
    fine = ((core * NSUP + T_sup) * 2 + half) * KSUP + ti
    NFINE = NCORES * NSUP * 2 * KSUP
    counts = np.bincount(fine, minlength=NFINE)
    CH = int(np.ceil(counts.max() / 128))
    SEG = CH * 128

    order = np.argsort(fine, kind="stable")
    fine_s = fine[order]
    offs = np.concatenate([[0], np.cumsum(counts)])
    pos = np.arange(E) - np.repeat(offs[:-1], counts)

    # flat slot layout per core: segments (T, h) with ti-major inner blocks;
    # fine group (T, h, ti) occupies slots [seg_base(T,h) + ti*SEG, ...+cnt).
    seg_tiles = []
    for T in range(NSUP):
        for h in (0, 1):
            seg_tiles.append(SUP_TILES[T])
    seg_slot_off = np.concatenate([[0], np.cumsum([t * SEG for t in seg_tiles])])
    SLTOT = int(seg_slot_off[-1])  # slots per core

    # fine group -> global slot base
    fine_base = np.zeros(NFINE, dtype=np.int64)
    for c in range(NCORES):
        for T in range(NSUP):
            for h in (0, 1):
                s = T * 2 + h
                for ti_ in range(SUP_TILES[T]):
                    f = ((c * NSUP + T) * 2 + h) * KSUP + ti_
                    fine_base[f] = c * SLTOT + seg_slot_off[s] + ti_ * SEG

    slot = fine_base[fine_s] + pos  # global slot per sorted edge

    big_idx = np.full(NCORES * SLTOT, 0, dtype=np.int16)
    big_dl = np.full(NCORES * SLTOT, 999.0, dtype=np.float32)
    big_idx[slot] = idx16[order]
    big_dl[slot] = dloc[order].astype(np.float32)
    big_idx = big_idx.reshape(NCORES, SLTOT)
    big_dl = big_dl.reshape(NCORES, SLTOT)

    # degrees
    cnt = np.bincount(dst, minlength=N).astype(np.float32)
    inv = 1.0 / np.maximum(cnt, 1.0)

    # weights (shared)
    BF = ml_dtypes.bfloat16
    Wmap_pad = np.zeros((F_IN_PAD, 128), np.float32)
    Wmap_pad[0:F_IN] = W_map
    wmap_kt = np.concatenate([Wmap_pad[_ts(k)] for k in range(4)], axis=1)
    Wl3_pad = np.zeros((D2, NCLS_PAD), np.float32)
    Wl3_pad[:, 0:NCLS] = Wl3
    wl3_kt = np.concatenate([Wl3_pad[_ts(k)] for k in range(2)], axis=1)
    Wr3_pad = np.zeros((D2, NCLS_PAD), np.float32)
    Wr3_pad[:, 0:NCLS] = Wr3
    wr3_kt = np.concatenate([Wr3_pad[_ts(k)] for k in range(2)], axis=1)
    bl3_pad = np.zeros((NCLS_PAD, 1), np.float32)
    bl3_pad[0:NCLS, 0] = bl3

    shared = {
        "iota": np.ascontiguousarray(
            np.tile(np.arange(128, dtype=np.float32), (128, 1))).astype(BF),
        "ident": np.eye(128, dtype=np.float32).astype(BF),
        "ident32": np.eye(128, dtype=np.float32),
        "wmap": np.ascontiguousarray(wmap_kt).astype(BF),
        "bmap": np.ascontiguousarray(b_map.reshape(128, 1)),
        "wl1": np.ascontiguousarray(Wl1).astype(BF),
        "wr1": np.ascontiguousarray(Wr1).astype(BF),
        "bl1": np.ascontiguousarray(bl1.reshape(128, 1)),
        "wl2": np.ascontiguousarray(Wl2).astype(BF),
        "wr2": np.ascontiguousarray(Wr2).astype(BF),
        "bl2": np.ascontiguousarray(bl2.reshape(2, 128).T),
        "wl3": np.ascontiguousarray(wl3_kt).astype(BF),
        "wr3": np.ascontiguousarray(wr3_kt).astype(BF),
        "bl3": bl3_pad,
    }

    in_maps = []
    for c in range(NCORES):
        xT_pad = np.zeros((F_IN_PAD, NLOC_PAD), np.float32)
        xT_pad[0:F_IN, 0:NLOC] = x[c * NLOC:(c + 1) * NLOC].T
        xT_pad = xT_pad.astype(ml_dtypes.bfloat16)

        # idx layout: per (T,h) segment packed independently, concat cols
        seg_cols = []
        dl_cols = []
        for T in range(NSUP):
            for h in (0, 1):
                s = T * 2 + h
                a, b = int(seg_slot_off[s]), int(seg_slot_off[s + 1])
                vals = big_idx[c, a:b]
                seg_cols.append(_pack_idx_segment(vals))
                dls = big_dl[c, a:b].reshape(-1, 128).T  # [128, tiles*CH]
                dl_cols.append(dls)
        idx_arr = np.ascontiguousarray(np.concatenate(seg_cols, axis=1))
        dl_arr = np.ascontiguousarray(
            np.concatenate(dl_cols, axis=1)).astype(BF)

        inv_pad = np.ones(NLOC_PAD, np.float32)
        inv_pad[0:NLOC] = inv[c * NLOC:(c + 1) * NLOC]
        invdeg_arr = np.ascontiguousarray(
            np.broadcast_to(inv_pad, (128, NLOC_PAD)))

        m = {
            "xT": xT_pad,
            "idx": idx_arr,
            "dl": dl_arr,
            "invdeg": invdeg_arr,
        }
        m.update(shared)
        in_maps.append(m)
    return in_maps, CH


_prog_cache = {}


def kernel(**inputs) -> np.ndarray:
    args = {k: np.asarray(v) for k, v in inputs.items()}
    in_maps, CH = prepare_inputs(
        args["x"], args["edge_index"], args["W_map"], args["b_map"],
        args["Wl1"], args["bl1"], args["Wr1"], args["Wl2"], args["bl2"],
        args["Wr2"], args["Wl3"], args["bl3"], args["Wr3"])

    if CH not in _prog_cache:
        _prog_cache[CH] = build_program(CH)
    nc = _prog_cache[CH]

    trace = os.environ.get("KERNEL_TRACE", "0") == "1"
    kw = {}
    if trace:
        import concourse.bass_utils as bu
        bu.upload_artifacts = lambda t: ""
        kw = dict(trace=True, tmpdir=os.environ.get(
            "KERNEL_TRACE_DIR", "/tmp/kernel_trace"))
    res = run_bass_kernel_spmd(nc, in_maps, list(range(NCORES)), **kw)
    if trace and res.exec_time_ns is not None:
        print(f"HW exec time: {res.exec_time_ns} ns")

    out = np.concatenate([res.results[c]["out"] for c in range(NCORES)], axis=0)
    return out.astype(np.float32)
